# revision 1
# baseline (speedup 1.0000x reference)
"""GCNNet on 8 TRN2 NeuronCores — full network on device.

Strategy (dst-sharded, 12500 nodes/core, relabeled by degree within shard):
  * hs = dinv * (h @ W) computed densely per shard (PE), AllGather -> hs_full.
  * Sparse aggregation: per-edge rows of hs_full fetched with dma_gather
    (padded-degree layout: per (tile, src-block) compile-time capacity taken
    from the actual graph), segment-reduced on the vector engine.
    norm trick: norm = dinv[src]*dinv[dst] =>
    agg_i = dinv_i * sum_j (dinv_j * hw_j) + 2*dinv_i^2*hw_i + b.
  * fc1 + graph pooling via one-hot S matmuls into PSUM windows,
    dma_scatter_add (unique rows) into absolute graph rows, AllReduce,
    fc3 + log_softmax on device. Host only builds int16 index streams.

Falls back to a pure-numpy path if the input violates the layout assumptions
(>512 graphs per shard etc.).
"""
import numpy as np

N_NODES = 100000
N_EDGES = 3200000
N_GRAPHS = 2000
IN_F, DIM, GDIM, OUT = 37, 32, 128, 2
NC = 8
SH = N_NODES // NC          # 12500
NT = 98                      # tiles per core
PAD = NT * 128               # 12544 padded nodes per core
BLK = 2 * PAD                # 25088 rows per src block (int16-addressable)
NBLK = 4
HS_W = 64                    # hs row width in f32 (256B, dma_gather elem floor)
GRP = 4                      # tiles per gather group
ZROW = SH                    # block-local guaranteed-zero row (first pad row)
PROWS = 2688                 # pooled rows (2048 real+pad, >=2112 dummy)
GT_HEAD = 2048               # graph rows processed by the head

_cache = {"inp_key": None, "prep": None, "nc": None, "sched_key": None}


# ---------------------------------------------------------------- host prep
def _preprocess(x, edge_index, batch):
    src = edge_index[0].astype(np.int32)
    dst = edge_index[1].astype(np.int32)
    deg = np.bincount(dst, minlength=N_NODES)
    dinv = (deg + 2.0) ** -0.5

    # relabel nodes within each shard by max per-src-block in-edge count:
    # block membership of a src is invariant under within-shard relabeling,
    # so sorting dst nodes by max_b cnt(n, b) directly tightens the
    # per-(tile, block) gather capacities (35% fewer padded tokens than
    # sorting by total degree).
    blk_src = (src // SH) // 2
    cnt_nb = np.bincount(dst.astype(np.int64) * 4 + blk_src,
                         minlength=N_NODES * 4).reshape(N_NODES, 4)
    maxblk = cnt_nb.max(axis=1)
    order = np.argsort(-maxblk.reshape(NC, SH), axis=1, kind="stable")
    rank = np.empty((NC, SH), np.int64)
    np.put_along_axis(rank, order, np.broadcast_to(np.arange(SH), (NC, SH)), 1)
    newpos = ((np.arange(N_NODES, dtype=np.int32) // SH) * PAD
              + rank.reshape(-1).astype(np.int32))

    drow = newpos[dst]
    srow = newpos[src]
    blk = srow // BLK
    ival = (srow % BLK).astype(np.int16)
    key = (drow * 4 + blk).astype(np.int32)
    o = np.argsort(key, kind="stable")
    ks = key[o]
    first = np.empty(N_EDGES, bool)
    first[0] = True
    np.not_equal(ks[1:], ks[:-1], out=first[1:])
    gstart = np.flatnonzero(first)
    gid = np.cumsum(first) - 1
    slot = np.arange(N_EDGES) - gstart[gid]

    cnt = np.bincount(key, minlength=NC * PAD * 4).reshape(NC, NT, 128, 4)
    Dtb = cnt.max(axis=2)                   # [NC, NT, 4]
    sched = Dtb.max(axis=0)                 # [NT, 4] shared across cores (SPMD)

    # groups of GRP tiles; capacity padded to group max
    gsizes = [GRP] * (NT // GRP) + ([NT % GRP] if NT % GRP else [])
    ngrp = len(gsizes)
    Dgb = np.zeros((ngrp, 4), np.int64)
    for g in range(ngrp):
        t0 = g * GRP
        Dgb[g] = sched[t0:t0 + gsizes[g]].max(axis=0)
    Dgb = np.maximum(Dgb, 1)
    opoff = np.zeros((ngrp, 4), np.int64)
    off = 0
    for g in range(ngrp):
        for b in range(4):
            opoff[g, b] = off
            off += 128 * gsizes[g] * Dgb[g, b]
    ltok = off

    dr = drow[o]
    core_e = dr // PAD
    loc = dr % PAD
    t_e = loc // 128
    p_e = loc % 128
    b_e = ks % 4
    g_e = t_e // GRP
    tg_e = t_e % GRP
    Dg_e = Dgb[g_e, b_e]
    pos = opoff[g_e, b_e] + (tg_e * Dg_e + slot) * 128 + p_e

    streams = np.full((NC, ltok), ZROW, np.int16)
    streams[core_e, pos] = ival[o]
    gidx_w = np.ascontiguousarray(
        streams.reshape(NC, ltok // 16, 16).transpose(0, 2, 1))

    # per-node tiled arrays in new order
    ar_nc = np.arange(NC)[:, None]
    dinv_new = np.zeros((NC, PAD), np.float32)  # pads 0 -> hs pad rows forced to 0
    dinv_new[ar_nc, rank] = dinv.reshape(NC, SH).astype(np.float32)
    dinv_t = np.ascontiguousarray(
        dinv_new.reshape(NC, NT, 128).transpose(0, 2, 1))

    batch = batch.astype(np.int64)
    if np.any(np.diff(batch) < 0):
        return None          # batch must be sorted for pooling windows
    g_lo = batch.reshape(NC, SH)[:, 0]
    g_hi = batch.reshape(NC, SH)[:, -1]
    if np.any(g_hi - g_lo >= 384):
        return None          # pooling windows insufficient -> fallback
    batch_new = np.full((NC, PAD), 1.0e6, np.float32)
    batch_new[ar_nc, rank] = (batch.reshape(NC, SH) - g_lo[:, None]).astype(np.float32)
    batchr_t = np.ascontiguousarray(
        batch_new.reshape(NC, NT, 128).transpose(0, 2, 1))

    pool_idx = np.empty((NC, 384), np.int64)
    for c in range(NC):
        j = np.arange(384)
        absr = g_lo[c] + j
        pool_idx[c] = np.where(absr < GT_HEAD, absr, 2112 + j)
    pool_idx_w = np.ascontiguousarray(np.tile(
        pool_idx.astype(np.int16).reshape(NC, 24, 16).transpose(0, 2, 1),
        (1, 8, 1)))

    import ml_dtypes
    xp = np.zeros((NC, PAD, IN_F), np.float32)
    xp[ar_nc, rank] = x.reshape(NC, SH, IN_F)
    xT = np.ascontiguousarray(xp.transpose(0, 2, 1)).astype(ml_dtypes.bfloat16)

    sched_key = (ltok, tuple(gsizes), tuple(map(tuple, Dgb.tolist())))
    return dict(gidx_w=gidx_w, dinv_t=dinv_t, batchr_t=batchr_t,
                pool_idx_w=pool_idx_w, xT=xT, ltok=ltok, gsizes=gsizes,
                Dgb=Dgb, opoff=opoff, sched_key=sched_key)


# ---------------------------------------------------------------- device
def _build_bass(ltok, gsizes, Dgb, opoff):
    import concourse.bacc as bacc
    import concourse.tile as tile
    from concourse import mybir

    f32 = mybir.dt.float32
    bf16 = mybir.dt.bfloat16
    fp8 = mybir.dt.float8e4
    i16 = mybir.dt.int16
    AT = mybir.ActivationFunctionType
    AL = mybir.AluOpType
    AX = mybir.AxisListType
    ngrp = len(gsizes)
    L16 = ltok // 16

    nc = bacc.Bacc("TRN2", target_bir_lowering=False, debug=False,
                   num_devices=NC, num_swdge_queues=4)
    # inputs
    xT = nc.dram_tensor("xT", [IN_F, PAD], bf16, kind="ExternalInput").ap()
    gidx = nc.dram_tensor("gidx", [16, L16], i16, kind="ExternalInput").ap()
    dinvt = nc.dram_tensor("dinvt", [128, NT], f32, kind="ExternalInput").ap()
    batchrt = nc.dram_tensor("batchrt", [128, NT], f32, kind="ExternalInput").ap()
    pidx = nc.dram_tensor("pidx", [128, 24], i16, kind="ExternalInput").ap()
    w1 = nc.dram_tensor("w1", [IN_F, DIM], bf16, kind="ExternalInput").ap()
    w2 = nc.dram_tensor("w2", [DIM, DIM], bf16, kind="ExternalInput").ap()
    w3 = nc.dram_tensor("w3", [DIM, DIM], bf16, kind="ExternalInput").ap()
    fc1w = nc.dram_tensor("fc1w", [3 * DIM + 1, GDIM], bf16, kind="ExternalInput").ap()
    fc3w = nc.dram_tensor("fc3w", [GDIM, OUT], f32, kind="ExternalInput").ap()
    b1b = nc.dram_tensor("b1b", [128, DIM], f32, kind="ExternalInput").ap()
    b2b = nc.dram_tensor("b2b", [128, DIM], f32, kind="ExternalInput").ap()
    b3b = nc.dram_tensor("b3b", [128, DIM], f32, kind="ExternalInput").ap()
    fc3b = nc.dram_tensor("fc3b", [128, OUT], f32, kind="ExternalInput").ap()
    out = nc.dram_tensor("out", [GT_HEAD, OUT], f32, kind="ExternalOutput").ap()
    # scratch
    hs_local = nc.dram_tensor("hs_local", [NT, 128, HS_W], f32, kind="Internal").ap()
    hs_full = nc.dram_tensor("hs_full", [NC * PAD, HS_W], f32, kind="Internal").ap()
    gidx_rep = nc.dram_tensor("gidx_rep", [128, L16], i16, kind="Internal").ap()
    pooled = nc.dram_tensor("pooled", [PROWS, GDIM], f32, kind="Internal").ap()
    pooledr = nc.dram_tensor("pooledr", [PROWS, GDIM], f32, kind="Internal").ap()

    with tile.TileContext(nc) as tc:
        with (
            tc.tile_pool(name="const", bufs=1) as cp,
            tc.tile_pool(name="big", bufs=1) as bigp,
            tc.tile_pool(name="gat", bufs=2) as gatp,
            tc.tile_pool(name="gidxp", bufs=3) as gip,
            tc.tile_pool(name="small", bufs=3) as smp,
            tc.tile_pool(name="ps", bufs=2, space="PSUM") as psp,
            tc.tile_pool(name="pspool", bufs=1, space="PSUM") as pspool,
        ):
            # ---- constants to SBUF
            def cload(ap_, shape, dt, tag):
                t = cp.tile(shape, dt, tag=tag)
                nc.sync.dma_start(t[:], ap_[:])
                return t
            xsb = cp.tile([128, PAD], bf16, tag="xsb")
            nc.sync.dma_start(xsb[0:IN_F, :], xT[:])
            w1s = cp.tile([128, DIM], bf16, tag="w1s")
            nc.sync.dma_start(w1s[0:IN_F, :], w1[:])
            w2s = cp.tile([128, DIM], bf16, tag="w2s")
            nc.sync.dma_start(w2s[0:DIM, :], w2[:])
            w3s = cp.tile([128, DIM], bf16, tag="w3s")
            nc.sync.dma_start(w3s[DIM:2 * DIM, :], w3[:])
            fc1ws = cp.tile([128, GDIM], bf16, tag="fc1ws")
            nc.sync.dma_start(fc1ws[0:3 * DIM + 1, :], fc1w[:])
            fc3ws = cload(fc3w, [GDIM, OUT], f32, "fc3ws")
            b1s = cload(b1b, [128, DIM], f32, "b1s")
            b2s = cload(b2b, [128, DIM], f32, "b2s")
            b3s = cload(b3b, [128, DIM], f32, "b3s")
            fc3bs = cload(fc3b, [128, OUT], f32, "fc3bs")
            ioti = cp.tile([128, 384], mybir.dt.int32, tag="ioti")
            nc.gpsimd.iota(ioti[:], [[1, 384]], base=0, channel_multiplier=0)
            iotas = cp.tile([128, 384], f32, tag="iotas")
            nc.vector.tensor_copy(iotas[:], ioti[:])
            iotd = cp.tile([128, 128], mybir.dt.int32, tag="iotd")
            nc.gpsimd.iota(iotd[:], [[1, 128]], base=0, channel_multiplier=-1)
            idents = cp.tile([128, 128], f32, tag="idents")
            nc.vector.tensor_scalar(idents[:], iotd[:], 0, None, AL.is_equal)
            dinvs = cload(dinvt, [128, NT], f32, "dinvs")
            batchrs = cload(batchrt, [128, NT], f32, "batchrs")
            pidxs = cload(pidx, [128, 24], i16, "pidxs")

            # persistent big buffers
            hcat = bigp.tile([128, PAD], bf16, tag="hcat")
            xlT = bigp.tile([128, PAD], bf16, tag="xlT")
            nc.vector.memset(hcat[96:128, :], 1.0)  # ones row for fc1 bias
            hsbig = bigp.tile([128, NT, HS_W], f32, tag="hsbig")
            aggacc = bigp.tile([128, NT, DIM], f32, tag="aggacc")
            dinvbig = bigp.tile([128, NT, DIM], f32, tag="dinvbig")
            nc.vector.tensor_copy(
                dinvbig[:, :, :],
                dinvs[:].unsqueeze(-1).broadcast_to([128, NT, DIM]))

            # replicate gather indices to all 8 gpsimd core stripes
            for k in range(8):
                nc.sync.dma_start(gidx_rep[16 * k:16 * (k + 1), :], gidx[:])
            # zero pooled
            zt = cp.tile([128, GDIM], f32, tag="zt")
            nc.vector.memset(zt[:], 0.0)
            for k in range(PROWS // 128):
                nc.sync.dma_start(pooled[128 * k:128 * (k + 1), :], zt[:])

            wmat = [w1s, w2s, w3s]
            bmat = [b1s, b2s, b3s]
            qctr = [0]

            for layer in range(3):
                # ---- dense: hs_local = dinv * (h @ W)
                nc.vector.memset(hsbig[:, :, :], 0.0)
                for t in range(NT):
                    mm = psp.tile([128, DIM], f32, tag="mm")
                    if layer == 0:
                        lhsT = xsb[0:IN_F, t * 128:(t + 1) * 128]
                        rhs = w1s[0:IN_F, :]
                    else:
                        lhsT = hcat[DIM * (layer - 1):DIM * layer,
                                    t * 128:(t + 1) * 128]
                        rhs = wmat[layer][DIM * (layer - 1):DIM * layer, :]
                    nc.tensor.matmul(mm[:], lhsT, rhs,
                                     start=True, stop=True)
                    nc.scalar.activation(hsbig[:, t, 0:DIM], mm[:], AT.Copy,
                                         scale=dinvs[:, t:t + 1])
                nc.sync.dma_start(hs_local[:].transpose([1, 0, 2]),
                                  hsbig[:, :, :])
                # ---- AllGather shards
                nc.gpsimd.collective_compute(
                    "AllGather", mybir.AluOpType.bypass,
                    replica_groups=[list(range(NC))],
                    ins=[hs_local[:]], outs=[hs_full[:]])
                # ---- gather + segment reduce
                for g in range(ngrp):
                    gs = gsizes[g]
                    t0 = g * GRP
                    gcols = gs * int(sum(Dgb[g]))
                    goff16 = int(opoff[g][0]) // 16
                    it = gip.tile([128, 8 * gcols], i16, tag="it")
                    nc.sync.dma_start(
                        it[:], gidx_rep[:, goff16:goff16 + 8 * gcols])
                    for b in range(4):
                        dg = int(Dgb[g][b])
                        cols = gs * dg
                        cb = gs * int(sum(Dgb[g][:b]))
                        gt = gatp.tile([128, cols, HS_W], f32, tag="gt")
                        # SWDGE ring caps ~1024 descriptors per op: chunk the
                        # gather into <=8-column (1024-idx) sub-ops rotated
                        # over the 4 queues.
                        for j0 in range(0, cols, 8):
                            cc = min(8, cols - j0)
                            nc.gpsimd.dma_gather(
                                gt[:, j0:j0 + cc, :],
                                hs_full[b * BLK:(b + 1) * BLK, :],
                                it[:, 8 * (cb + j0):8 * (cb + j0 + cc)],
                                128 * cc, 128 * cc, HS_W,
                                queue_num=qctr[0] % 4)
                            qctr[0] += 1
                        v = gt[:, :, 0:DIM].rearrange(
                            "p (g d) f -> p g f d", g=gs)
                        dst_sl = aggacc[:, t0:t0 + gs, :]
                        if b == 0:
                            nc.vector.tensor_reduce(
                                out=dst_sl, in_=v, op=AL.add, axis=AX.X)
                        else:
                            rt = smp.tile([128, GRP, DIM], f32, tag="rt")
                            nc.vector.tensor_reduce(
                                out=rt[:, 0:gs, :], in_=v, op=AL.add, axis=AX.X)
                            nc.vector.tensor_tensor(
                                out=dst_sl, in0=dst_sl, in1=rt[:, 0:gs, :],
                                op=AL.add)
                # ---- epilogue: x_l = [relu](dinv*(agg + 2*hs) + b)
                nc.vector.tensor_scalar(hsbig[:, :, 0:DIM],
                                        hsbig[:, :, 0:DIM], 2.0,
                                        None, AL.mult)
                nc.vector.tensor_tensor(aggacc[:, :, :], aggacc[:, :, :],
                                        hsbig[:, :, 0:DIM], AL.add)
                nc.vector.tensor_tensor(aggacc[:, :, :], aggacc[:, :, :],
                                        dinvbig[:, :, :], AL.mult)
                nc.vector.tensor_tensor(
                    aggacc[:, :, :], aggacc[:, :, :],
                    bmat[layer][:].unsqueeze(1).broadcast_to([128, NT, DIM]),
                    AL.add)
                if layer == 0:
                    nc.vector.tensor_scalar(aggacc[:, :, :], aggacc[:, :, :],
                                            0.0, None, AL.max)
                # ---- transpose x_l into hcat rows
                for t in range(NT):
                    tp = psp.tile([128, 128], f32, tag="mm")
                    nc.tensor.transpose(tp[0:DIM, :], aggacc[:, t, 0:DIM],
                                        idents[:])
                    if layer == 0:
                        nc.scalar.copy(
                            hcat[0:DIM, t * 128:(t + 1) * 128], tp[0:DIM, :])
                    else:
                        nc.scalar.copy(
                            xlT[0:DIM, t * 128:(t + 1) * 128], tp[0:DIM, :])
                if layer > 0:
                    nc.sync.dma_start(
                        hcat[DIM * layer:DIM * (layer + 1), :], xlT[0:DIM, :])

            # ---- fc1 + pooling windows
            pps = []
            for w in range(3):
                pw = pspool.tile([128, 128], f32, tag=f"pw{w}")
                pps.append(pw)
            for t in range(NT):
                mmf = psp.tile([128, GDIM], f32, tag="mm")
                nc.tensor.matmul(
                    mmf[:], hcat[0:3 * DIM + 1, t * 128:(t + 1) * 128],
                    fc1ws[0:3 * DIM + 1, :], start=True, stop=True)
                h2 = smp.tile([128, GDIM], f32, tag="h2")
                nc.vector.tensor_scalar(h2[:], mmf[:], 0.0, None, AL.max)
                S = smp.tile([128, 384], f32, tag="S")
                nc.vector.tensor_scalar(S[:], iotas[:], batchrs[:, t:t + 1],
                                        None, AL.is_equal)
                for w in range(3):
                    nc.tensor.matmul(pps[w][:], S[:, 128 * w:128 * (w + 1)],
                                     h2[:], start=(t == 0), stop=(t == NT - 1),
                                     skip_group_check=True)
            pl = cp.tile([128, 3, GDIM], f32, tag="pl")
            for w in range(3):
                nc.vector.tensor_copy(pl[:, w, :], pps[w][:])
            nc.gpsimd.dma_scatter_add(pooled[:, :], pl[:, :, :], pidxs[:],
                                      384, 384, GDIM)
            # ---- AllReduce pooled, head
            nc.gpsimd.collective_compute(
                "AllReduce", mybir.AluOpType.add,
                replica_groups=[list(range(NC))],
                ins=[pooled[:]], outs=[pooledr[:]])
            for k in range(GT_HEAD // 128):
                pt = smp.tile([128, GDIM], f32, tag="pt")
                nc.sync.dma_start(pt[:], pooledr[128 * k:128 * (k + 1), :])
                trp = psp.tile([128, 128], f32, tag="mm")
                nc.tensor.transpose(trp[:], pt[:], idents[:])
                ptT = smp.tile([128, 128], f32, tag="ptT")
                nc.vector.tensor_copy(ptT[:], trp[:])
                mml = psp.tile([128, OUT], f32, tag="mm")
                nc.tensor.matmul(mml[:], ptT[:], fc3ws[:], start=True, stop=True)
                lg = smp.tile([128, OUT], f32, tag="lg")
                nc.vector.tensor_tensor(lg[:], mml[:], fc3bs[:], AL.add)
                m = smp.tile([128, 1], f32, tag="m")
                nc.vector.tensor_reduce(out=m[:], in_=lg[:], op=AL.max, axis=AX.X)
                nm = smp.tile([128, 1], f32, tag="nm")
                nc.vector.tensor_scalar(nm[:], m[:], -1.0, None, AL.mult)
                e = smp.tile([128, OUT], f32, tag="e")
                s = smp.tile([128, 1], f32, tag="s")
                nc.scalar.activation(e[:], lg[:], AT.Exp, bias=nm[:],
                                     scale=1.0, accum_out=s[:])
                ls = smp.tile([128, 1], f32, tag="ls")
                nc.scalar.activation(ls[:], s[:], AT.Ln)
                nls = smp.tile([128, 1], f32, tag="nls")
                nc.vector.tensor_scalar(nls[:], ls[:], -1.0, None, AL.mult)
                og = smp.tile([128, OUT], f32, tag="og")
                nc.vector.tensor_scalar(og[:], lg[:], nm[:], nls[:],
                                        AL.add, AL.add)
                nc.sync.dma_start(out[128 * k:128 * (k + 1), :], og[:])
    nc.compile()
    return nc


# ---------------------------------------------------------------- fallback
def _kernel_numpy(x, edge_index, batch, w1, b1, w2, b2, w3, b3,
                  fc1_w, fc1_b, fc3_w, fc3_b):
    src = edge_index[0].astype(np.int64)
    dst = edge_index[1].astype(np.int64)
    batch = batch.astype(np.int64)
    n = max(N_NODES, int(dst.max()) + 1, int(src.max()) + 1,
            x.shape[0])
    deg = np.bincount(dst, minlength=n).astype(np.float32) + 2.0
    dinv = deg ** -0.5
    norm = (dinv[src] * dinv[dst]).astype(np.float32)
    try:
        import scipy.sparse as sp
        A = sp.csr_matrix((norm, (dst, src)), shape=(n, n))
        spmv = lambda h: np.asarray(A @ h, np.float32)
    except ImportError:
        def spmv(h):
            out = np.zeros((n, h.shape[1]), np.float32)
            np.add.at(out, dst, h[src] * norm[:, None])
            return out
    self_w = (2.0 * dinv * dinv)[:, None].astype(np.float32)

    def conv(h, W, bb):
        hw = h @ np.asarray(W, np.float32)
        return spmv(hw) + self_w * hw + np.asarray(bb, np.float32)

    x1 = np.maximum(conv(np.asarray(x, np.float32), w1, b1), 0.0)
    x2 = conv(x1, w2, b2)
    x3 = conv(x2, w3, b3)
    h = np.concatenate([x1, x2, x3], axis=1)
    h = np.maximum(h @ np.asarray(fc1_w, np.float32) + fc1_b, 0.0)
    pooled = np.zeros((int(batch.max()) + 1, GDIM), np.float32)
    np.add.at(pooled, batch, h)
    pooled = pooled[:N_GRAPHS] if pooled.shape[0] >= N_GRAPHS else np.pad(
        pooled, ((0, N_GRAPHS - pooled.shape[0]), (0, 0)))
    logits = pooled @ np.asarray(fc3_w, np.float32) + fc3_b
    m = logits.max(axis=1, keepdims=True)
    lse = m + np.log(np.exp(logits - m).sum(axis=1, keepdims=True))
    return (logits - lse).astype(np.float32)


# ---------------------------------------------------------------- entry
def kernel(x, edge_index, batch, w1, b1, w2, b2, w3, b3,
           fc1_w, fc1_b, fc3_w, fc3_b):
    global _cache
    x = np.asarray(x, np.float32)
    edge_index = np.asarray(edge_index)
    batch_np = np.asarray(batch)
    if (x.shape != (N_NODES, IN_F) or edge_index.shape != (2, N_EDGES)
            or batch_np.shape != (N_NODES,)):
        return _kernel_numpy(x, edge_index, batch_np, w1, b1, w2, b2, w3, b3,
                             fc1_w, fc1_b, fc3_w, fc3_b)

    try:
        if (_cache["inp_key"] is not None
                and np.array_equal(_cache["inp_key"][0], edge_index)
                and np.array_equal(_cache["inp_key"][1], batch_np)
                and np.array_equal(_cache["inp_key"][2], x)):
            prep = _cache["prep"]
        else:
            prep = _preprocess(x, edge_index, batch_np)
            _cache["inp_key"] = (edge_index.copy(), batch_np.copy(), x.copy())
            _cache["prep"] = prep
        if prep is None:
            return _kernel_numpy(x, edge_index, batch_np, w1, b1, w2, b2,
                                 w3, b3, fc1_w, fc1_b, fc3_w, fc3_b)

        if _cache["nc"] is None or _cache["sched_key"] != prep["sched_key"]:
            _cache["nc"] = _build_bass(prep["ltok"], prep["gsizes"],
                                       prep["Dgb"].tolist(),
                                       prep["opoff"].tolist())
            _cache["sched_key"] = prep["sched_key"]
        nc = _cache["nc"]
        return _run_device(nc, prep, w1, b1, w2, b2, w3, b3,
                           fc1_w, fc1_b, fc3_w, fc3_b)
    except Exception as exc:          # device path failed -> host fallback
        import sys
        print(f"kernel: device path failed ({exc!r}); numpy fallback",
              file=sys.stderr)
        _cache["nc"] = None
        _cache["sched_key"] = None
        return _kernel_numpy(x, edge_index, batch_np, w1, b1, w2, b2, w3, b3,
                             fc1_w, fc1_b, fc3_w, fc3_b)


def _run_device(nc, prep, w1, b1, w2, b2, w3, b3, fc1_w, fc1_b, fc3_w, fc3_b):

    import ml_dtypes
    bf = ml_dtypes.bfloat16
    w1b = np.ascontiguousarray(np.asarray(w1, np.float32)).astype(bf)
    w2b = np.ascontiguousarray(np.asarray(w2, np.float32)).astype(bf)
    w3b = np.ascontiguousarray(np.asarray(w3, np.float32)).astype(bf)
    fc1wb = np.ascontiguousarray(np.vstack(
        [np.asarray(fc1_w, np.float32),
         np.asarray(fc1_b, np.float32)[None, :]])).astype(bf)
    fc3wf = np.ascontiguousarray(np.asarray(fc3_w, np.float32))

    def bias_pad(b):
        return np.broadcast_to(
            np.asarray(b, np.float32), (128, DIM)).copy()
    b1p, b2p, b3p = bias_pad(b1), bias_pad(b2), bias_pad(b3)
    fc3bp = np.broadcast_to(np.asarray(fc3_b, np.float32), (128, OUT)).copy()

    in_maps = []
    for c in range(NC):
        in_maps.append(dict(
            xT=prep["xT"][c], gidx=prep["gidx_w"][c], dinvt=prep["dinv_t"][c],
            batchrt=prep["batchr_t"][c], pidx=prep["pool_idx_w"][c],
            w1=w1b, w2=w2b, w3=w3b, fc1w=fc1wb, fc3w=fc3wf,
            b1b=b1p, b2b=b2p, b3b=b3p, fc3b=fc3bp))

    import os
    from concourse.bass_utils import run_bass_kernel_spmd
    trace = bool(os.environ.get("KERNEL_TRACE"))
    res = run_bass_kernel_spmd(nc, in_maps, list(range(NC)), trace=trace)
    global last_results
    last_results = res
    return np.ascontiguousarray(res.results[0]["out"][:N_GRAPHS])

